# revision 5
# baseline (speedup 1.0000x reference)
"""Trainium2 Bass kernel for nn_AggrSum (segment_sum of H rows by X_node).

out[v, :] = sum_{n : X_node[n] == v} H[n, :],  H [1600000, 128] f32,
X_node [1600000] int64 in [0, 100000).

Strategy (8 NeuronCores, SPMD single program):
  * Host planning: argsort X_node; the V axis is tiled into PSEG-segment
    sub-windows dealt round-robin by size to (core, slot) so a single
    SPMD program with per-slot chunk counts covers all cores.  Each
    core's rows are packed host-side into its input tensor in the exact
    SBUF layout the kernel consumes ([128 partitions, TOT, 128] fp16).
    fp16 (2 B/elem) halves HBM traffic vs fp32; quantization error is
    ~5e-4 relative, far under the 2e-2 gate.
  * Device: gather groups of ~GCH chunks stream via one large DMA,
    alternating between the sync and scalar HWDGE rings so completion
    latency on one ring hides under streaming on the other; ONE
    is_equal per group (DVE, every third on GPSIMD) builds the one-hot
    selection block [128, nch*PSEG] from a resident [128, PSEG] iota
    tile read with a repeat-block AP; per 128-row chunk one matmul
    (lhsT=onehot [128, PSEG], rhs=H chunk [128, 128]) accumulates a
    [PSEG, 128] f32 PSUM slice.  SUBS consecutive sub-windows land at
    partition offsets 0/64 of one [128, 128] PSUM tile (PE
    tile_position), so output stays full width.  ACT copies PSUM ->
    fp16 staging; batched output DMAs go out on the scalar HWDGE ring.
  * Host scatters the per-core sub-window blocks back to V order.

Segment-sharded output means no cross-core reduction; each core
streams ~1/8 of the rows once (~55 MB) and writes 3.2 MB.
"""
import dataclasses

import numpy as np

import concourse.bass as bass
import concourse.mybir as mybir
import concourse.tile as tile
from concourse import bacc
from concourse import bass_utils

P = 128
D = 128
PSEG = 64
SUBS = P // PSEG          # sub-windows per PSUM tile
OUT_BATCH = 8             # PSUM tiles per output staging DMA
GCH = 120                 # target chunks per gather DMA (~3.9 MB)
ENG_PATTERN = "v"         # one-hot engine per gather group
N_CORES = 8
F32 = mybir.dt.float32
F16 = mybir.dt.float16

_CACHE = {}


def _plan_schedule(X, n_cores):
    N = X.shape[0]
    V = int(X.max()) + 1 if N else 1
    perm = np.argsort(X)
    Xs = X[perm].astype(np.int64)

    nws = -(-V // PSEG)
    W = -(-nws // n_cores)
    W4 = -(-W // SUBS) * SUBS
    NW = W4 * n_cores

    win_of_node = Xs // PSEG
    counts = np.bincount(win_of_node, minlength=NW)[:NW]
    starts = np.zeros(NW + 1, dtype=np.int64)
    np.cumsum(counts, out=starts[1:])

    wsorted = np.argsort(-counts, kind="stable")
    assign = wsorted.reshape(W4, n_cores)
    Ks = np.maximum(1, -(-counts[assign].max(axis=1) // P)).astype(np.int64)
    off = np.zeros(W4 + 1, dtype=np.int64)
    np.cumsum(Ks, out=off[1:])
    TOT = int(off[-1])

    # gather groups: consecutive slots, <= GCH chunks each
    groups = []
    s0 = 0
    while s0 < W4:
        s1 = s0
        while s1 < W4 and off[s1 + 1] - off[s0] <= GCH:
            s1 += 1
        groups.append((s0, s1, int(off[s0]), int(off[s1])))
        s0 = s1
    gmax = max(t1 - t0 for _, _, t0, t1 in groups)

    order = np.full((n_cores, TOT * P), -1, dtype=np.int64)
    xrel = np.full((n_cores, P, TOT), -1.0, dtype=np.float32)
    for c in range(n_cores):
        ov = order[c].reshape(TOT, P)
        xr = xrel[c]
        for s in range(W4):
            gwi = int(assign[s, c])
            a, b = int(starts[gwi]), int(starts[gwi + 1])
            cnt = b - a
            if cnt == 0:
                continue
            t0 = int(off[s])
            nch = (cnt + P - 1) // P
            ov[t0:t0 + nch].ravel()[:cnt] = perm[a:b]
            tmp = np.full(nch * P, -1.0, np.float32)
            tmp[:cnt] = (Xs[a:b] - gwi * PSEG).astype(np.float32)
            xr[:, t0:t0 + nch] = tmp.reshape(-1, P).T

    iota = np.ascontiguousarray(np.broadcast_to(
        np.arange(PSEG, dtype=np.float16)[None, :], (P, PSEG)))

    return dict(
        V=V, W4=W4, Ks=Ks, off=off, TOT=TOT, n_cores=n_cores,
        groups=groups, gmax=gmax, assign=assign, order=order,
        xrel=xrel.astype(np.float16), iota=iota,
    )


def _make_in_maps(H, meta):
    n_cores, TOT = meta["n_cores"], meta["TOT"]
    H16 = H.astype(np.float16)
    maps = []
    for c in range(n_cores):
        flat = meta["order"][c]
        sel = np.clip(flat, 0, None)
        hh = H16[sel]
        hh[flat < 0] = 0
        hh = np.ascontiguousarray(hh.reshape(TOT, P, D).transpose(1, 0, 2))
        maps.append({
            "h": hh,
            "xrel": meta["xrel"][c],
            "iota": meta["iota"],
        })
    return maps


def _assemble_output(res_outs, meta):
    n_cores, W4, V = meta["n_cores"], meta["W4"], meta["V"]
    assign = meta["assign"]
    full = np.zeros((-(-V // PSEG) * PSEG + W4 * PSEG, D), dtype=np.float32)
    for c in range(n_cores):
        # out layout: [P, NGOUT*D] fp16; partition p = PSEG*(s%SUBS)+q
        oc = res_outs[c].astype(np.float32).reshape(P, W4 // SUBS, D)
        oc = oc.reshape(SUBS, PSEG, W4 // SUBS, D)
        for s in range(W4):
            gwi = int(assign[s, c])
            full[gwi * PSEG:(gwi + 1) * PSEG] = oc[s % SUBS, :, s // SUBS]
    return full[:V]


def _rep_block(ap, n):
    # [part, [1, b]] -> [part, [0, n], [1, b]]  (repeat the block n times)
    part = ap.ap[0]
    new = [part, [0, n], list(ap.ap[1])]
    return dataclasses.replace(ap, ap=new)


def _rep_elem(ap, block):
    # [part, [s, n]] -> [part, [s, n], [0, block]]
    part = ap.ap[0]
    new = [part, list(ap.ap[1]), [0, block]]
    return dataclasses.replace(ap, ap=new)


def _build_nc(W4, Ks, off, groups, gmax, n_cores, nbufs=4):
    Ks = [int(k) for k in Ks]
    off = [int(o) for o in off]
    TOT = off[-1]
    NGOUT = W4 // SUBS
    nc = bacc.Bacc("TRN2", target_bir_lowering=False, debug=False,
                   num_devices=n_cores)
    h = nc.dram_tensor("h", [P, TOT, D], F16, kind="ExternalInput").ap()
    xrel_d = nc.dram_tensor("xrel", [P, TOT], F16, kind="ExternalInput").ap()
    iota_d = nc.dram_tensor("iota", [P, PSEG], F16,
                            kind="ExternalInput").ap()
    out_d = nc.dram_tensor("out", [P, NGOUT * D], F16,
                           kind="ExternalOutput").ap()

    with tile.TileContext(nc) as tc:
        with (
            tc.tile_pool(name="res", bufs=1) as res,
            tc.tile_pool(name="gat", bufs=nbufs) as gat,
            tc.tile_pool(name="oh", bufs=3) as ohp,
            tc.tile_pool(name="ps", bufs=4, space="PSUM") as ps,
            tc.tile_pool(name="osb", bufs=2) as osb,
        ):
            xrel_sb = res.tile([P, TOT], F16)
            iota_sb = res.tile([P, PSEG], F16)
            nc.sync.dma_start(out=xrel_sb[:], in_=xrel_d[:])
            nc.sync.dma_start(out=iota_sb[:], in_=iota_d[:])

            pt = None
            staging = None
            for gi, (s0, s1, t0, t1) in enumerate(groups):
                nch = t1 - t0
                gt = gat.tile([P, nch * D], F16, tag="gt")
                geng = nc.sync if gi % 2 == 0 else nc.scalar
                geng.dma_start(
                    out=gt[:],
                    in_=h[:, t0:t1, :].rearrange("p t d -> p (t d)"))
                oh = ohp.tile([P, nch * PSEG], F16, tag="oh")
                oeng = (nc.vector
                        if ENG_PATTERN[gi % len(ENG_PATTERN)] == "v"
                        else nc.gpsimd)
                oeng.tensor_tensor(
                    out=oh[:],
                    in0=_rep_block(iota_sb[:], nch),
                    in1=_rep_elem(xrel_sb[:, t0:t1], PSEG),
                    op=mybir.AluOpType.is_equal,
                )
                for s in range(s0, s1):
                    i = s % SUBS
                    if i == 0:
                        pt = ps.tile([P, D], F32, tag="pt")
                    rt = off[s] - t0
                    K = Ks[s]
                    for j in range(K):
                        nc.tensor.matmul(
                            out=pt[PSEG * i:PSEG * (i + 1), :],
                            lhsT=oh[:, (rt + j) * PSEG:(rt + j + 1) * PSEG],
                            rhs=gt[:, (rt + j) * D:(rt + j + 1) * D],
                            start=(j == 0), stop=(j == K - 1),
                            tile_position=(0, PSEG * i),
                        )
                    if i == SUBS - 1:
                        og = s // SUBS
                        b = og % OUT_BATCH
                        if b == 0:
                            staging = osb.tile([P, OUT_BATCH * D], F16,
                                               tag="st")
                        nc.scalar.copy(out=staging[:, b * D:(b + 1) * D],
                                       in_=pt[:])
                        if b == OUT_BATCH - 1 or og == NGOUT - 1:
                            g0 = og - b
                            nc.scalar.dma_start(
                                out=out_d[:, g0 * D:(og + 1) * D],
                                in_=staging[:, :(b + 1) * D])

    nc.compile()
    return nc


def prepare(H, X_node):
    """Plan + build + shard. Returns (nc, in_maps, meta). Cached on the
    schedule signature so repeated kernel() calls reuse the compiled
    program."""
    H = np.ascontiguousarray(np.asarray(H, dtype=np.float32))
    X = np.asarray(X_node).astype(np.int64)
    assert H.ndim == 2 and H.shape[1] == D and X.shape == (H.shape[0],)

    meta = _plan_schedule(X, N_CORES)
    key = (meta["W4"], tuple(int(k) for k in meta["Ks"]))
    if key not in _CACHE:
        _CACHE[key] = _build_nc(meta["W4"], meta["Ks"], meta["off"],
                                meta["groups"], meta["gmax"], N_CORES)
    nc = _CACHE[key]
    in_maps = _make_in_maps(H, meta)
    return nc, in_maps, meta


def kernel(H, X_node):
    nc, in_maps, meta = prepare(H, X_node)
    res = bass_utils.run_bass_kernel_spmd(
        nc, in_maps, core_ids=list(range(N_CORES)))
    out = _assemble_output([res.results[c]["out"] for c in range(N_CORES)],
                           meta)
    return out.astype(np.float32)


# revision 6
# speedup vs baseline: 1.0998x; 1.0998x over previous
"""Trainium2 Bass kernel for nn_AggrSum (segment_sum of H rows by X_node).

out[v, :] = sum_{n : X_node[n] == v} H[n, :],  H [1600000, 128] f32,
X_node [1600000] int64 in [0, 100000).

Strategy (8 NeuronCores, SPMD single program):
  * Host planning: argsort X_node; the V axis is tiled into PSEG-segment
    sub-windows dealt round-robin by size to (core, slot) so a single
    SPMD program with per-slot chunk counts covers all cores.  Each
    core's rows are packed host-side into its input tensor in the exact
    SBUF layout the kernel consumes ([128 partitions, TOT, 128] fp16).
    fp16 (2 B/elem) halves HBM traffic vs fp32; quantization error is
    ~5e-4 relative, far under the 2e-2 gate.
  * Device: gather groups of ~GCH chunks stream via one large DMA,
    alternating between the sync and scalar HWDGE rings so completion
    latency on one ring hides under streaming on the other; ONE
    is_equal per group (DVE, every third on GPSIMD) builds the one-hot
    selection block [128, nch*PSEG] from a resident [128, PSEG] iota
    tile read with a repeat-block AP; per 128-row chunk one matmul
    (lhsT=onehot [128, PSEG], rhs=H chunk [128, 128]) accumulates a
    [PSEG, 128] f32 PSUM slice.  SUBS consecutive sub-windows land at
    partition offsets 0/64 of one [128, 128] PSUM tile (PE
    tile_position), so output stays full width.  ACT copies PSUM ->
    fp16 staging; batched output DMAs go out on the scalar HWDGE ring.
  * Host scatters the per-core sub-window blocks back to V order.

Segment-sharded output means no cross-core reduction; each core
streams ~1/8 of the rows once (~55 MB) and writes 3.2 MB.
"""
import dataclasses

import numpy as np

import concourse.bass as bass
import concourse.mybir as mybir
import concourse.tile as tile
from concourse import bacc
from concourse import bass_utils

P = 128
D = 128
PSEG = 64
SUBS = P // PSEG          # sub-windows per PSUM tile
OUT_BATCH = 8             # PSUM tiles per output staging DMA
GCH = 64                  # target chunks per gather DMA (~2.1 MB)
ENG_PATTERN = "v"         # one-hot engine per gather group
N_CORES = 8
F32 = mybir.dt.float32
F16 = mybir.dt.float16

_CACHE = {}


def _plan_schedule(X, n_cores):
    N = X.shape[0]
    V = int(X.max()) + 1 if N else 1
    perm = np.argsort(X)
    Xs = X[perm].astype(np.int64)

    nws = -(-V // PSEG)
    W = -(-nws // n_cores)
    W4 = -(-W // SUBS) * SUBS
    NW = W4 * n_cores

    win_of_node = Xs // PSEG
    counts = np.bincount(win_of_node, minlength=NW)[:NW]
    starts = np.zeros(NW + 1, dtype=np.int64)
    np.cumsum(counts, out=starts[1:])

    wsorted = np.argsort(-counts, kind="stable")
    assign = wsorted.reshape(W4, n_cores)
    Ks = np.maximum(1, -(-counts[assign].max(axis=1) // P)).astype(np.int64)
    off = np.zeros(W4 + 1, dtype=np.int64)
    np.cumsum(Ks, out=off[1:])
    TOT = int(off[-1])

    # gather groups: consecutive slots, <= GCH chunks each
    groups = []
    s0 = 0
    while s0 < W4:
        s1 = s0
        while s1 < W4 and off[s1 + 1] - off[s0] <= GCH:
            s1 += 1
        groups.append((s0, s1, int(off[s0]), int(off[s1])))
        s0 = s1
    gmax = max(t1 - t0 for _, _, t0, t1 in groups)

    order = np.full((n_cores, TOT * P), -1, dtype=np.int64)
    xrel = np.full((n_cores, P, TOT), -1.0, dtype=np.float32)
    for c in range(n_cores):
        ov = order[c].reshape(TOT, P)
        xr = xrel[c]
        for s in range(W4):
            gwi = int(assign[s, c])
            a, b = int(starts[gwi]), int(starts[gwi + 1])
            cnt = b - a
            if cnt == 0:
                continue
            t0 = int(off[s])
            nch = (cnt + P - 1) // P
            ov[t0:t0 + nch].ravel()[:cnt] = perm[a:b]
            tmp = np.full(nch * P, -1.0, np.float32)
            tmp[:cnt] = (Xs[a:b] - gwi * PSEG).astype(np.float32)
            xr[:, t0:t0 + nch] = tmp.reshape(-1, P).T

    iota = np.ascontiguousarray(np.broadcast_to(
        np.arange(PSEG, dtype=np.float16)[None, :], (P, PSEG)))

    return dict(
        V=V, W4=W4, Ks=Ks, off=off, TOT=TOT, n_cores=n_cores,
        groups=groups, gmax=gmax, assign=assign, order=order,
        xrel=xrel.astype(np.float16), iota=iota,
    )


def _make_in_maps(H, meta):
    n_cores, TOT = meta["n_cores"], meta["TOT"]
    H16 = H.astype(np.float16)
    maps = []
    for c in range(n_cores):
        flat = meta["order"][c]
        sel = np.clip(flat, 0, None)
        hh = H16[sel]
        hh[flat < 0] = 0
        hh = np.ascontiguousarray(hh.reshape(TOT, P, D).transpose(1, 0, 2))
        maps.append({
            "h": hh,
            "xrel": meta["xrel"][c],
            "iota": meta["iota"],
        })
    return maps


def _assemble_output(res_outs, meta):
    n_cores, W4, V = meta["n_cores"], meta["W4"], meta["V"]
    assign = meta["assign"]
    full = np.zeros((-(-V // PSEG) * PSEG + W4 * PSEG, D), dtype=np.float32)
    for c in range(n_cores):
        # out layout: [P, NGOUT*D] fp16; partition p = PSEG*(s%SUBS)+q
        oc = res_outs[c].astype(np.float32).reshape(P, W4 // SUBS, D)
        oc = oc.reshape(SUBS, PSEG, W4 // SUBS, D)
        for s in range(W4):
            gwi = int(assign[s, c])
            full[gwi * PSEG:(gwi + 1) * PSEG] = oc[s % SUBS, :, s // SUBS]
    return full[:V]


def _rep_block(ap, n):
    # [part, [1, b]] -> [part, [0, n], [1, b]]  (repeat the block n times)
    part = ap.ap[0]
    new = [part, [0, n], list(ap.ap[1])]
    return dataclasses.replace(ap, ap=new)


def _rep_elem(ap, block):
    # [part, [s, n]] -> [part, [s, n], [0, block]]
    part = ap.ap[0]
    new = [part, list(ap.ap[1]), [0, block]]
    return dataclasses.replace(ap, ap=new)


def _build_nc(W4, Ks, off, groups, gmax, n_cores, nbufs=6):
    Ks = [int(k) for k in Ks]
    off = [int(o) for o in off]
    TOT = off[-1]
    NGOUT = W4 // SUBS
    nc = bacc.Bacc("TRN2", target_bir_lowering=False, debug=False,
                   num_devices=n_cores)
    h = nc.dram_tensor("h", [P, TOT, D], F16, kind="ExternalInput").ap()
    xrel_d = nc.dram_tensor("xrel", [P, TOT], F16, kind="ExternalInput").ap()
    iota_d = nc.dram_tensor("iota", [P, PSEG], F16,
                            kind="ExternalInput").ap()
    out_d = nc.dram_tensor("out", [P, NGOUT * D], F16,
                           kind="ExternalOutput").ap()

    with tile.TileContext(nc) as tc:
        with (
            tc.tile_pool(name="res", bufs=1) as res,
            tc.tile_pool(name="gat", bufs=nbufs) as gat,
            tc.tile_pool(name="oh", bufs=6) as ohp,
            tc.tile_pool(name="ps", bufs=4, space="PSUM") as ps,
            tc.tile_pool(name="osb", bufs=2) as osb,
        ):
            xrel_sb = res.tile([P, TOT], F16)
            iota_sb = res.tile([P, PSEG], F16)
            nc.gpsimd.dma_start(out=xrel_sb[:], in_=xrel_d[:])
            nc.gpsimd.dma_start(out=iota_sb[:], in_=iota_d[:])

            pt = None
            staging = None
            for gi, (s0, s1, t0, t1) in enumerate(groups):
                nch = t1 - t0
                gt = gat.tile([P, nch * D], F16, tag="gt")
                geng = nc.sync if gi % 2 == 0 else nc.scalar
                geng.dma_start(
                    out=gt[:],
                    in_=h[:, t0:t1, :].rearrange("p t d -> p (t d)"))
                oh = ohp.tile([P, nch * PSEG], F16, tag="oh")
                oeng = (nc.vector
                        if ENG_PATTERN[gi % len(ENG_PATTERN)] == "v"
                        else nc.gpsimd)
                oeng.tensor_tensor(
                    out=oh[:],
                    in0=_rep_block(iota_sb[:], nch),
                    in1=_rep_elem(xrel_sb[:, t0:t1], PSEG),
                    op=mybir.AluOpType.is_equal,
                )
                for s in range(s0, s1):
                    i = s % SUBS
                    if i == 0:
                        pt = ps.tile([P, D], F32, tag="pt")
                    rt = off[s] - t0
                    K = Ks[s]
                    for j in range(K):
                        nc.tensor.matmul(
                            out=pt[PSEG * i:PSEG * (i + 1), :],
                            lhsT=oh[:, (rt + j) * PSEG:(rt + j + 1) * PSEG],
                            rhs=gt[:, (rt + j) * D:(rt + j + 1) * D],
                            start=(j == 0), stop=(j == K - 1),
                            tile_position=(0, PSEG * i),
                        )
                    if i == SUBS - 1:
                        og = s // SUBS
                        b = og % OUT_BATCH
                        if b == 0:
                            staging = osb.tile([P, OUT_BATCH * D], F16,
                                               tag="st")
                        nc.scalar.copy(out=staging[:, b * D:(b + 1) * D],
                                       in_=pt[:])
                        if b == OUT_BATCH - 1 or og == NGOUT - 1:
                            g0 = og - b
                            nc.scalar.dma_start(
                                out=out_d[:, g0 * D:(og + 1) * D],
                                in_=staging[:, :(b + 1) * D])

    nc.compile()
    return nc


def prepare(H, X_node):
    """Plan + build + shard. Returns (nc, in_maps, meta). Cached on the
    schedule signature so repeated kernel() calls reuse the compiled
    program."""
    H = np.ascontiguousarray(np.asarray(H, dtype=np.float32))
    X = np.asarray(X_node).astype(np.int64)
    assert H.ndim == 2 and H.shape[1] == D and X.shape == (H.shape[0],)

    meta = _plan_schedule(X, N_CORES)
    key = (meta["W4"], tuple(int(k) for k in meta["Ks"]))
    if key not in _CACHE:
        _CACHE[key] = _build_nc(meta["W4"], meta["Ks"], meta["off"],
                                meta["groups"], meta["gmax"], N_CORES)
    nc = _CACHE[key]
    in_maps = _make_in_maps(H, meta)
    return nc, in_maps, meta


def kernel(H, X_node):
    nc, in_maps, meta = prepare(H, X_node)
    res = bass_utils.run_bass_kernel_spmd(
        nc, in_maps, core_ids=list(range(N_CORES)))
    out = _assemble_output([res.results[c]["out"] for c in range(N_CORES)],
                           meta)
    return out.astype(np.float32)
